# revision 44
# baseline (speedup 1.0000x reference)
"""Trainium2 Bass kernel for the GRU encoder-decoder model.

Model (see harness reference): B=1024, T=100, PRED=30, E=512, H=1024, IN=2.
  emb = tanh(obs @ We.T + be)                      (B,T,512)
  enc1 = GRU(emb), enc2 = GRU(enc1, h0=h_enc1)     hidden 1024
  out0 = enc2[:,-2] @ Wout.T + bout
  30-step autoregressive decoder with two GRU cells sharing one hidden.

Strategy: data-parallel over batch on 8 cores (128 rows/core), all compute in
a feature-on-partition layout ([128 part = feature chunk, free = batch]),
weights pre-transposed on the host as the stationary fp16 matmul operand.

Key structure decisions (from trace analysis):
  * gx1 = emb@Wih1.T (+all biases) precomputed in a N=512 phase, DRAM fp16.
  * gx2 = h1@Wih2.T (+all biases) is fused INTO the enc1 scan: after each
    step's recurrence matmuls the tensor engine computes gx2 for the hidden
    just produced, filling the gate-chain bubble and keeping the PE warm.
    The PSUM->SBUF copy applies the gx2 biases (per-tile tensor_scalar_add
    on Vector, which has slack in that phase).
  * No bias matmuls anywhere: bhn is applied by a per-bank u = ghn + bhn
    copy (Scalar for the encoder scans, Vector tensor_scalar_add in the
    decoder) emitted as soon as the ghn PSUM bank closes; the decoder's
    rzb is the sigmoid's activation bias and bin rides a fused
    scalar_tensor_tensor in t2.
  * r/z matmuls run as two half-waves into a bufs=2 PSUM pool with the
    ghn block between them, so accumulation groups close mid-step and the
    next step's waves only wait on early sigmoid banks.
  * Decoder emission order: dec1(t+1)'s hidden-side waves + ghn are
    emitted before step t's out block + demb, covering the chain2 /
    out->demb serial path; dec1's x-side closes wave 0 first.
  * dec1's weights prefetch via DMA drip through phase D's steps.
  * Gate chain at hidden-chunk granularity, smallest chunk first, with the
    r/z sigmoids interleaved between chunks; the next step's r/z matmuls
    (k-outer) consume hidden chunks as they emerge.
  * dec2 reads the same vector for input and hidden, so its r/z input +
    hidden weights are pre-summed on the host (saves 1/3 of its matmuls).
"""

import numpy as np
import ml_dtypes

import concourse.bass as bass
import concourse.mybir as mybir
import concourse.tile as tile
from concourse import bacc
from concourse.bass_utils import run_bass_kernel_spmd

F32 = mybir.dt.float32
BF16 = mybir.dt.bfloat16
FP16 = mybir.dt.float16
AF = mybir.ActivationFunctionType
ALU = mybir.AluOpType

N_CORES = 8
B, T, PRED = 1024, 100, 30
E, H, IN = 512, 1024, 2
BL = B // N_CORES          # 128 batch rows per core
G = 3 * H                  # 3072 stacked gate rows
KH = H // 128              # 8 hidden k-tiles
KE = E // 128              # 4 embedding k-tiles
NT = T * BL // 512         # 25 n-chunks of 512 in the gx1 phase
# chain chunks over the 8 hidden tiles: smallest first so the next scan
# step's matmuls (which consume h k-tiles in ascending order) start early.
CHUNKS = [(0, 1), (1, 2), (2, 4), (4, 6), (6, 8)]
# r/z m-tiles bank-interleaved [r0 r1 z0 z1 | r2 r3 z2 z3 | ...] so the
# sigmoid of bank j//2 unlocks chain chunk j.
P16 = [0, 1, 8, 9, 2, 3, 10, 11, 4, 5, 12, 13, 6, 7, 14, 15]

_CACHE = {}


def _rz_sl(lo, hi, zgate):
    """rz_sb slice holding r (or z) tiles [lo,hi) in the perm layout."""
    idx = [4 * (j // 2) + 2 * zgate + (j % 2) for j in range(lo, hi)]
    assert idx == list(range(idx[0], idx[0] + len(idx))), idx
    return slice(idx[0], idx[0] + len(idx))


# ----------------------------------------------------------------------------
# device program
# ----------------------------------------------------------------------------

def _emit_gx1_phase(nc, tc, wk, rhs_fn, gx_dram, gxb_s, pools):
    """gx1' = Wih1.T-tiles @ emb (+bias) -> DRAM fp16, software-pipelined:
    the next chunk's emb matmuls+tanh are slotted mid-way through this
    chunk's gx matmuls so the tensor engine never waits on the tanh."""
    ps_gx = pools["ps_gx"]
    gxop = pools["gxo"]
    pend = [rhs_fn(0)]
    for c in range(NT):
        rhs = pend.pop(0)   # list of [128, 512] APs, one per k-tile
        for a in range(24):
            if a == 6 and c + 1 < NT:
                pend.append(rhs_fn(c + 1))
            pg = ps_gx.tile([128, 512], F32, tag="pgx")
            for j in range(KE):
                nc.tensor.matmul(
                    pg[:], wk[:, j, a * 128:(a + 1) * 128], rhs[j],
                    start=(j == 0), stop=(j == KE - 1))
            gxo = gxop.tile([128, 4, 128], FP16, tag="gxo")
            nc.scalar.activation(
                gxo.rearrange("p t b -> p (t b)"), pg[:], AF.Identity,
                bias=gxb_s[:, a:a + 1])
            nc.sync.dma_start(
                out=gx_dram[4 * c:4 * c + 4, a].rearrange("t p b -> p t b"),
                in_=gxo[:])


def _emit_out_block(nc, wout_s, bout_s, hb, ps_pool, ps_tag, outp,
                    preds=None, t=None):
    """outT = h @ Wout.T + bout -> ([2,128] f32, [2,128] fp16)."""
    po = ps_pool.tile([2, 128], F32, tag=ps_tag)
    for j in range(KH):
        nc.tensor.matmul(po[:], wout_s[:, j, :], hb[:, j, :],
                         start=(j == 0), stop=(j == KH - 1))
    outf = outp.tile([2, 128], F32, tag="outf")
    nc.vector.tensor_scalar_add(outf[:], po[:], bout_s[:, 0:1])
    outb = outp.tile([2, 128], FP16, tag="outb")
    nc.vector.tensor_copy(outb[:], outf[:])
    if preds is not None:
        nc.sync.dma_start(out=preds[:, t, :], in_=outf[:])
    return outf, outb


def _emit_chain(nc, pools, p_ghn, hb, rz_sb, bhn_c, t2_emit, sig_cbs,
                u_vec=False, upd_gp=False):
    """Gate math after the matmuls: returns the new fp16 hidden state.

    t1 = (ghn + bhn) * r is a single fused scalar_tensor_tensor per hidden
    tile (bhn_c is a [128, 8] f32 column table), so bhn never costs a PE
    matmul.  rz_sb is filled lazily via sig_cbs[bank] (emitted just before
    the first chunk that needs that bank, so the r/z sigmoids interleave
    with the tanh chain on the Scalar queue).
    t2_emit(lo, hi, t1, t2) -> emits t2 = t1 + x-projection for the chunk.
    """
    tmp = pools["tmp"]
    hbp = pools["hb"]
    u = tmp.tile([128, 8, 128], FP16, tag="u", bufs=1)
    t1 = tmp.tile([128, 8, 128], FP16, tag="t1")
    t2 = tmp.tile([128, 8, 128], FP16, tag="t2", bufs=1)
    nn = tmp.tile([128, 8, 128], FP16, tag="nn", bufs=1)
    d = tmp.tile([128, 8, 128], FP16, tag="d", bufs=1)
    hz = tmp.tile([128, 8, 128], FP16, tag="hz", bufs=1)
    nhb = hbp.tile([128, 8, 128], FP16, tag="hb")
    done = set()
    udone = set()
    # software-pipelined emission: chunk c's h-update (d/hz/nhb) is emitted
    # AFTER chunk c+1's t1/t2/tanh, so the in-order Vector queue never has a
    # tanh-dependent op blocking the next chunk's independent t1.
    # u = ghn + bhn runs on Scalar per ghn PSUM bank as soon as that bank
    # closes, so the p_ghn banks are drained mid-chain (not gated on the
    # late sigmoids) and the pool's next allocation never stalls the PE.
    for i in range(len(CHUNKS) + 1):
        if i < len(CHUNKS):
            lo, hi = CHUNKS[i]
            for bk in range(lo // 2, (hi + 1) // 2):
                if bk not in done:
                    sig_cbs(bk)
                    done.add(bk)
            for gb in {lo // 4, (hi - 1) // 4}:
                if gb not in udone:
                    for a in range(4 * gb, 4 * gb + 4):
                        if u_vec:
                            nc.vector.tensor_scalar_add(
                                u[:, a, :], p_ghn[:, a, :],
                                bhn_c[:, a:a + 1])
                        else:
                            nc.scalar.activation(u[:, a, :], p_ghn[:, a, :],
                                                 AF.Identity,
                                                 bias=bhn_c[:, a:a + 1])
                    udone.add(gb)
            sl = slice(lo, hi)
            nc.vector.tensor_mul(t1[:, sl, :], u[:, sl, :],
                                 rz_sb[:, _rz_sl(lo, hi, 0), :])
            t2_emit(lo, hi, t1, t2)
            nc.scalar.activation(nn[:, sl, :], t2[:, sl, :], AF.Tanh)
        if i > 0:
            lo, hi = CHUNKS[i - 1]
            sl = slice(lo, hi)
            upd = nc.gpsimd if upd_gp else nc.vector
            upd.tensor_sub(d[:, sl, :], hb[:, sl, :], nn[:, sl, :])
            upd.tensor_mul(hz[:, sl, :], d[:, sl, :],
                           rz_sb[:, _rz_sl(lo, hi, 1), :])
            upd.tensor_add(nhb[:, sl, :], hz[:, sl, :], nn[:, sl, :])
    return nhb


def _emit_enc_scan(nc, tc, pools, whh_s, gx_dram, bhn_c, hb,
                   out98=None, gx2=None, dmas=None):
    """Encoder scan over T steps.

    gx2 (scan1 fusion): dict(w=wih2_s, b=gxb2_s cols, ps=psum pool, op=sbuf
    pool, dram=gx2_dram) — after each step's recurrence matmuls, compute
    gx2[t-1] = Wih2.T @ h1[t-1] on the tensor engine (6 waves of 4 m-tiles;
    the PSUM->SBUF copy folds the gxb2 biases on the Scalar engine).
    The r/z matmuls run as two half-waves into a bufs=2 PSUM pool, so the
    next step's wave 0 only waits for this step's bank-0/1 sigmoids (early
    in the chain) instead of all four.  bhn is applied in the chain's fused
    scalar_tensor_tensor — no bias matmuls anywhere.
    Returns (hb, out_res at t = T-2 if out98 given)."""
    gxp = pools["gx"]
    tmp = pools["tmp"]
    ps_rz, ps_ghn = pools["ps_rz"], pools["ps_ghn"]
    PF = 2
    pend = []
    for t in range(min(PF, T)):
        gxt = gxp.tile([128, 24, 128], FP16, tag="gxt")
        nc.sync.dma_start(out=gxt[:], in_=gx_dram[t].rearrange("a p b -> p a b"))
        pend.append(gxt)

    def emit_gx2(h_in, td):
        """gx2[td] = Wih2.T @ h1[td] + gxb2 -> DRAM (6 waves x 4 m-tiles)."""
        for w in range(6):
            pg = gx2["ps"].tile([128, 4, 128], F32, tag="pgx2")
            for j in range(KH):
                for a in range(4):
                    nc.tensor.matmul(
                        pg[:, a, :],
                        gx2["w"][:, j, (4 * w + a) * 128:(4 * w + a + 1) * 128],
                        h_in[:, j, :], start=(j == 0 and a == 0),
                        stop=(j == KH - 1 and a == 3))
            go = gx2["op"].tile([128, 4, 128], FP16, tag="gxo2")
            for a in range(4):
                nc.scalar.activation(go[:, a, :], pg[:, a, :], AF.Identity,
                                     bias=gx2["b"][:, 4 * w + a:4 * w + a + 1])
            nc.sync.dma_start(
                out=gx2["dram"][td, 4 * w:4 * w + 4].rearrange("a p b -> p a b"),
                in_=go[:])

    out_res = None
    for t in range(T):
        gxt = pend.pop(0)
        if dmas and t >= 4 and t % 2 == 0:
            dmas.pop(0)()
        h_in = hb
        rzs = tmp.tile([128, 16, 128], FP16, tag="rzs", bufs=1)
        rz_sb = tmp.tile([128, 16, 128], FP16, tag="rz")
        # k-outer so each MM burst consumes one hidden chunk as the previous
        # step's chain emits it; two half-waves of 8 m-tiles (2 PSUM banks
        # each, double-buffered) so wave 0 of step t+1 only waits on this
        # step's first two sigmoid banks.
        # emission order [w0][ghn][w1]: ghn's bank 0 closes mid-step, so the
        # chain (whose t1 reads are gated on the ghn accumulation group
        # CLOSING, not just the tiles being written) starts ~5us earlier and
        # its nhb chunks are ready before the next step's w0 bursts need
        # them — removes the exposed chain tail in the enc2 phase.
        w_tiles = []

        def rz_wave(w):
            pw = ps_rz.tile([128, 8, 128], F32, tag="prz")
            for j in range(KH):
                for ai in range(8):
                    a = 8 * w + ai
                    nc.tensor.matmul(
                        pw[:, ai, :], whh_s[:, j, a * 128:(a + 1) * 128],
                        hb[:, j, :], start=(j == 0 and ai % 4 == 0),
                        stop=(j == KH - 1 and ai % 4 == 3))
            w_tiles.append(pw)

        rz_wave(0)
        p_ghn = ps_ghn.tile([128, 8, 128], F32, tag="pghn")
        for a in range(8):
            for j in range(KH):
                nc.tensor.matmul(
                    p_ghn[:, a, :], whh_s[:, j, (16 + a) * 128:(17 + a) * 128],
                    hb[:, j, :], start=(a % 4 == 0 and j == 0),
                    stop=(a % 4 == 3 and j == KH - 1))
        rz_wave(1)

        def sig(bk, w_tiles=w_tiles, gxt=gxt, rzs=rzs, rz_sb=rz_sb):
            sl = slice(4 * bk, 4 * bk + 4)
            psl = slice(4 * (bk % 2), 4 * (bk % 2) + 4)
            nc.vector.tensor_add(rzs[:, sl, :], w_tiles[bk // 2][:, psl, :],
                                 gxt[:, sl, :])
            nc.scalar.activation(rz_sb[:, sl, :], rzs[:, sl, :], AF.Sigmoid)

        def t2_enc(lo, hi, t1, t2, gxt=gxt):
            nc.vector.tensor_add(t2[:, lo:hi, :], t1[:, lo:hi, :],
                                 gxt[:, 16 + lo:16 + hi, :])

        hb = _emit_chain(nc, pools, p_ghn, hb, rz_sb, bhn_c, t2_enc, sig)
        if gx2 is not None and t > 0:
            emit_gx2(h_in, t - 1)
        if out98 is not None and t == T - 2:
            wout_s, bout_s, outp = out98
            out_res = _emit_out_block(nc, wout_s, bout_s, hb, ps_ghn, "pghn",
                                      outp)
        if t + PF < T:
            gxt2 = gxp.tile([128, 24, 128], FP16, tag="gxt")
            nc.sync.dma_start(out=gxt2[:],
                              in_=gx_dram[t + PF].rearrange("a p b -> p a b"))
            pend.append(gxt2)
    if gx2 is not None:
        emit_gx2(hb, T - 1)
    return hb, out_res


def _dec_cell_parts(nc, pools, nkx, lhsT_fn, xn_ks, hb,
                    rzb_c, bin_c, bhn_c):
    """Emitter closures for one decoder GRU cell; the caller controls the
    PE emission order so PSUM accumulation groups CLOSE as early as
    possible (chain reads are gated on the group's stop matmul, not on the
    tiles being written) and so independent matmul blocks cover the serial
    out->demb->x path.  No bias matmuls: rzb is the sigmoid's activation
    bias, bin rides the fused t2 scalar_tensor_tensor, bhn the fused t1."""
    tmp = pools["tmp"]
    ps_rz, ps_ghn, ps_x = pools["ps_rz"], pools["ps_ghn"], pools["ps_gxn"]
    st = {"w": [], "gxn": None}

    def rz_wave_h(w):
        pw = ps_rz.tile([128, 8, 128], F32, tag="prz")
        st["w"].append(pw)
        for ji in range(KH):
            for ai in range(8):
                a = 8 * w + ai
                nc.tensor.matmul(pw[:, ai, :], lhsT_fn("rz", nkx + ji, a),
                                 hb[:, ji, :],
                                 start=(ji == 0 and ai % 4 == 0),
                                 stop=(nkx == 0 and ji == KH - 1 and ai % 4 == 3))

    def rz_wave_x(w, xrhs):
        pw = st["w"][w]
        for xi in range(nkx):
            for ai in range(8):
                a = 8 * w + ai
                nc.tensor.matmul(pw[:, ai, :], lhsT_fn("rz", xi, a),
                                 xrhs(xi), start=False,
                                 stop=(xi == nkx - 1 and ai % 4 == 3))

    def ghn():
        p_ghn = ps_ghn.tile([128, 8, 128], F32, tag="pghn")
        st["ghn"] = p_ghn
        for a in range(8):
            for j in range(KH):
                nc.tensor.matmul(p_ghn[:, a, :], lhsT_fn("hn", j, a),
                                 hb[:, j, :], start=(a % 4 == 0 and j == 0),
                                 stop=(a % 4 == 3 and j == KH - 1))

    def gxn_half(h, xn_rhs):
        if st["gxn"] is None:
            p_gxn = ps_x.tile([128, 8, 128], F32, tag="pgxn")
            st["gxn"] = p_gxn
        p_gxn = st["gxn"]
        for j in range(xn_ks):
            for a in range(4 * h, 4 * h + 4):
                nc.tensor.matmul(p_gxn[:, a, :], lhsT_fn("xn", j, a),
                                 xn_rhs(j), start=(j == 0 and a == 4 * h),
                                 stop=(j == xn_ks - 1 and a == 4 * h + 3))

    def chain():
        rz_sb = tmp.tile([128, 16, 128], FP16, tag="rz")

        def sig(bk):
            pw = st["w"][bk // 2]
            for i in range(4):
                a = 4 * bk + i
                nc.scalar.activation(rz_sb[:, a, :], pw[:, 4 * (bk % 2) + i, :],
                                     AF.Sigmoid, bias=rzb_c[:, a:a + 1])

        def t2_dec(lo, hi, t1, t2):
            for a in range(lo, hi):
                nc.vector.scalar_tensor_tensor(
                    t2[:, a, :], st["gxn"][:, a, :], bin_c[:, a:a + 1],
                    t1[:, a, :], ALU.add, ALU.add)

        return _emit_chain(nc, pools, st["ghn"], hb, rz_sb, bhn_c, t2_dec,
                           sig, u_vec=True)

    return rz_wave_h, rz_wave_x, ghn, gxn_half, chain


def build_program():
    nc = bacc.Bacc("TRN2", target_bir_lowering=False, debug=False,
                   num_devices=N_CORES)
    dp = nc.declare_dram_parameter
    obsT = dp("obsT", [2, T * BL], FP16, isOutput=False)
    WeT = dp("WeT", [2, E], FP16, isOutput=False)
    Wih1T = dp("Wih1T", [KE, 128, G], FP16, isOutput=False)
    Whh1T = dp("Whh1T", [KH, 128, G], FP16, isOutput=False)
    Wih2T = dp("Wih2T", [KH, 128, G], FP16, isOutput=False)
    Whh2T = dp("Whh2T", [KH, 128, G], FP16, isOutput=False)
    Wd1T = dp("Wd1T", [KE + KH, 128, G], FP16, isOutput=False)
    Wd2T = dp("Wd2T", [KH, 128, 4096], FP16, isOutput=False)
    WedT = dp("WedT", [2, E], FP16, isOutput=False)
    WoutT = dp("WoutT", [KH, 128, 2], FP16, isOutput=False)
    be_s = dp("be_s", [128, KE], F32, isOutput=False)
    gxb1 = dp("gxb1", [128, 24], F32, isOutput=False)
    gxb2 = dp("gxb2", [128, 24], F32, isOutput=False)
    bhn1_d = dp("bhn1_c", [128, KH], F32, isOutput=False)
    bhn2_d = dp("bhn2_c", [128, KH], F32, isOutput=False)
    d1_rzb_d = dp("d1_rzb_c", [128, 16], F32, isOutput=False)
    d1_bin_d = dp("d1_bin_c", [128, KH], F32, isOutput=False)
    d1_bhn_d = dp("d1_bhn_c", [128, KH], F32, isOutput=False)
    d2_rzb_d = dp("d2_rzb_c", [128, 16], F32, isOutput=False)
    d2_bin_d = dp("d2_bin_c", [128, KH], F32, isOutput=False)
    d2_bhn_d = dp("d2_bhn_c", [128, KH], F32, isOutput=False)
    bed_s = dp("bed_s", [128, KE], F32, isOutput=False)
    bout_s = dp("bout_s", [2, 1], F32, isOutput=False)
    preds = dp("preds", [2, PRED, BL], F32, isOutput=True)

    gx1 = nc.dram_tensor("gx1", [T, 24, 128, BL], FP16)
    gx2 = nc.dram_tensor("gx2", [T, 24, 128, BL], FP16)

    with tile.TileContext(nc) as tc:
        with tc.tile_pool(name="const", bufs=1) as constp, \
             tc.tile_pool(name="hbp", bufs=2) as hbp, \
             tc.tile_pool(name="outp", bufs=2) as outp:
            def cload(name, ap, shape, dtype=F32):
                t = constp.tile(shape, dtype, tag=name)
                nc.sync.dma_start(out=t[:], in_=ap)
                return t
            gxb1_s = cload("gxb1", gxb1[:], [128, 24])
            gxb2_s = cload("gxb2", gxb2[:], [128, 24])
            be_c = cload("be", be_s[:], [128, KE])
            bout_c = cload("bout", bout_s[:], [2, 1])
            bhn1_c = cload("bhn1c", bhn1_d[:], [128, KH])
            bhn2_c = cload("bhn2c", bhn2_d[:], [128, KH])

            pools = {"hb": hbp}

            # weights for the fused scan1 phase; loaded up-front so the DMAs
            # overlap phase A compute.
            with tc.tile_pool(name="wB", bufs=1) as wB:
                whh1_s = wB.tile([128, KH, G], FP16)
                for j in range(KH):
                    nc.sync.dma_start(out=whh1_s[:, j, :], in_=Whh1T[j])
                wih2_s = wB.tile([128, KH, G], FP16)
                for j in range(KH):
                    nc.sync.dma_start(out=wih2_s[:, j, :], in_=Wih2T[j])

                # ---------------- phase A: emb + gx1 ----------------
                with tc.tile_pool(name="wA", bufs=1) as wA, \
                     tc.tile_pool(name="sA", bufs=2) as sA, \
                     tc.tile_pool(name="embp", bufs=2) as embp, \
                     tc.tile_pool(name="gxoA", bufs=4) as gxoA, \
                     tc.tile_pool(name="psE", bufs=1, space="PSUM") as psE, \
                     tc.tile_pool(name="psGA", bufs=4, space="PSUM") as psGA:
                    weT_s = wA.tile([2, E], FP16)
                    nc.sync.dma_start(out=weT_s[:], in_=WeT[:])
                    wih1_s = wA.tile([128, KE, G], FP16)
                    for j in range(KE):
                        nc.sync.dma_start(out=wih1_s[:, j, :], in_=Wih1T[j])

                    # obs streamed in 5 double-buffered pieces of 5 chunks
                    # each (the whole [2, T*BL] tile would cost 25.6K/part).
                    obs_box = {}

                    def load_obs(i):
                        ot = sA.tile([2, 5 * 512], FP16, tag="obs")
                        nc.sync.dma_start(
                            out=ot[:], in_=obsT[:, i * 2560:(i + 1) * 2560])
                        obs_box[i] = ot

                    load_obs(0)

                    def rhs_emb(c):
                        i = c // 5
                        if c % 5 == 0 and i + 1 < 5 and i + 1 not in obs_box:
                            load_obs(i + 1)
                        obs_s = obs_box[i]
                        co = (c % 5) * 512
                        pe = psE.tile([128, 2, 512], F32, tag="pemb")
                        for et in range(2):
                            nc.tensor.matmul(
                                pe[:, et, :], weT_s[:, et * 128:(et + 1) * 128],
                                obs_s[:, co:co + 512],
                                start=True, stop=True)
                        pe2 = psE.tile([128, 2, 512], F32, tag="pemb2")
                        for et in range(2):
                            nc.tensor.matmul(
                                pe2[:, et, :], weT_s[:, (2 + et) * 128:(3 + et) * 128],
                                obs_s[:, co:co + 512],
                                start=True, stop=True)
                        embb = embp.tile([128, KE, 512], FP16, tag="emb")
                        for et in range(KE):
                            src = pe[:, et, :] if et < 2 else pe2[:, et - 2, :]
                            nc.scalar.activation(embb[:, et, :], src,
                                                 AF.Tanh, bias=be_c[:, et:et + 1])
                        return [embb[:, et, :] for et in range(KE)]
                    _emit_gx1_phase(nc, tc, wih1_s, rhs_emb, gx1, gxb1_s,
                                    {"ps_gx": psGA, "gxo": gxoA})

                # ---------------- phase B: enc1 scan + fused gx2 ------------
                # whh2 prefetches during B on the right-side pool stack
                # (independent LIFO), its k-tile DMAs dripped between steps.
                wDpre = tc.alloc_tile_pool(name="wDpre", bufs=1, side="right")
                whh2_s = wDpre.tile([128, KH, G], FP16)
                wout_s = wDpre.tile([128, KH, 2], FP16)
                dmasB = [
                    (lambda j=j: nc.sync.dma_start(out=whh2_s[:, j, :],
                                                   in_=Whh2T[j]))
                    for j in range(KH)]
                dmasB.append(lambda: nc.sync.dma_start(
                    out=wout_s[:], in_=WoutT.ap().rearrange("j p m -> p j m")))
                hb = hbp.tile([128, KH, 128], FP16, tag="hb")
                nc.vector.memset(hb[:], 0.0)
                with tc.tile_pool(name="gxB", bufs=3) as gxB, \
                     tc.tile_pool(name="tmpB", bufs=2) as tmpB, \
                     tc.tile_pool(name="gxo2", bufs=2) as gxo2, \
                     tc.tile_pool(name="psRZ", bufs=2, space="PSUM") as psRZ, \
                     tc.tile_pool(name="psGH", bufs=1, space="PSUM") as psGH, \
                     tc.tile_pool(name="psGX2", bufs=2, space="PSUM") as psGX2:
                    pls = dict(pools, gx=gxB, tmp=tmpB, ps_rz=psRZ, ps_ghn=psGH)
                    g2 = {"w": wih2_s, "b": gxb2_s, "ps": psGX2, "op": gxo2,
                          "dram": gx2}
                    hb, _ = _emit_enc_scan(nc, tc, pls, whh1_s, gx1, bhn1_c,
                                           hb, gx2=g2, dmas=dmasB)

            # ---------------- phase D: enc2 scan (+ out_loc0 at t=98) -------
            # dec1's weights + the small decoder constants load during D
            # (wE1 outlives the D pools; LIFO: released after phase E).
            wE1 = tc.alloc_tile_pool(name="wE1", bufs=1)
            wd1_s = wE1.tile([128, KE + KH, G], FP16)
            dmasD = [
                (lambda j=j: nc.sync.dma_start(out=wd1_s[:, j, :],
                                               in_=Wd1T[j]))
                for j in range(KE + KH)]
            wed_s = wE1.tile([2, E], FP16)
            nc.sync.dma_start(out=wed_s[:], in_=WedT[:])
            wout2_s = wE1.tile([128, KH, 2], FP16)
            nc.sync.dma_start(out=wout2_s[:],
                              in_=WoutT.ap().rearrange("j p m -> p j m"))
            rzb1_c = wE1.tile([128, 16], F32)
            nc.sync.dma_start(out=rzb1_c[:], in_=d1_rzb_d[:])
            bin1_c = wE1.tile([128, KH], F32)
            nc.sync.dma_start(out=bin1_c[:], in_=d1_bin_d[:])
            bhnd1_c = wE1.tile([128, KH], F32)
            nc.sync.dma_start(out=bhnd1_c[:], in_=d1_bhn_d[:])
            rzb2_c = wE1.tile([128, 16], F32)
            nc.sync.dma_start(out=rzb2_c[:], in_=d2_rzb_d[:])
            bin2_c = wE1.tile([128, KH], F32)
            nc.sync.dma_start(out=bin2_c[:], in_=d2_bin_d[:])
            bhnd2_c = wE1.tile([128, KH], F32)
            nc.sync.dma_start(out=bhnd2_c[:], in_=d2_bhn_d[:])
            bed_c = wE1.tile([128, KE], F32)
            nc.sync.dma_start(out=bed_c[:], in_=bed_s[:])
            with tc.tile_pool(name="gxD", bufs=3) as gxD, \
                 tc.tile_pool(name="tmpD", bufs=2) as tmpD, \
                 tc.tile_pool(name="psRZD", bufs=2, space="PSUM") as psRZD, \
                 tc.tile_pool(name="psGHD", bufs=2, space="PSUM") as psGHD:
                pls = dict(pools, gx=gxD, tmp=tmpD, ps_rz=psRZD, ps_ghn=psGHD)
                hb, out_res = _emit_enc_scan(
                    nc, tc, pls, whh2_s, gx2, bhn2_c, hb,
                    out98=(wout_s, bout_c, outp), dmas=dmasD)
                outf, outb = out_res
            wDpre.release()

            # ---------------- phase E: decoder ----------------
            with tc.tile_pool(name="wE2", bufs=1) as wE2, \
                 tc.tile_pool(name="dembp", bufs=2) as dembp, \
                 tc.tile_pool(name="tmpE", bufs=2) as tmpE, \
                 tc.tile_pool(name="psRZE", bufs=2, space="PSUM") as psRZE, \
                 tc.tile_pool(name="psGHE", bufs=1, space="PSUM") as psGHE, \
                 tc.tile_pool(name="psXE", bufs=1, space="PSUM") as psXE:
                wd2_s = wE2.tile([128, KH, 4096], FP16)
                for j in range(KH):
                    nc.sync.dma_start(out=wd2_s[:, j, :], in_=Wd2T[j])

                pls = dict(pools, tmp=tmpE, ps_rz=psRZE, ps_ghn=psGHE,
                           ps_gxn=psXE)
                demb_box = [None]

                def emit_demb(ob):
                    """demb = tanh(Wed @ out + bed), from the fp16 out."""
                    p_de = psXE.tile([128, KE, 128], F32, tag="pgxn")
                    for et in range(KE):
                        nc.tensor.matmul(
                            p_de[:, et, :], wed_s[:, et * 128:(et + 1) * 128],
                            ob[:], start=(et == 0), stop=(et == KE - 1))
                    demb = dembp.tile([128, KE, 128], FP16, tag="demb")
                    for et in range(KE):
                        nc.scalar.activation(demb[:, et, :], p_de[:, et, :],
                                             AF.Tanh, bias=bed_c[:, et:et + 1])
                    demb_box[0] = demb

                def l1h(part, j, a):
                    if part == "hn":
                        return wd1_s[:, KE + j, (16 + a) * 128:(17 + a) * 128]
                    m = a if part == "rz" else 16 + a
                    return wd1_s[:, j, m * 128:(m + 1) * 128]

                def l2(part, j, a):
                    off = {"rz": a * 128, "xn": 2048 + a * 128,
                           "hn": 3072 + a * 128}[part]
                    return wd2_s[:, j, off:off + 128]

                def d1_finish(d1):
                    """dec1 x-side: wave 0 closes first so sigmoid banks 0/1
                    and the chain start while gxn + wave 1 still stream."""
                    wh, wx, _, gxn, chain = d1
                    dembr = (lambda dd: lambda j: dd[:, j, :])(demb_box[0])
                    wh(1)
                    wx(0, dembr)
                    gxn(0, dembr)
                    gxn(1, dembr)
                    wx(1, dembr)
                    return chain()

                # loop structure per step: dec2's gxn banks close early (they
                # gate chain2's t2); dec1(t+1)'s w0h+ghn are emitted before
                # the out block + demb so the PE queue has cover while
                # chain2(t) resolves; dec1's w1h covers the demb tanh; the
                # x-side waves close wave 0 first to start chain1 early.
                emit_demb(outb)
                d1 = _dec_cell_parts(nc, pls, KE, l1h, KE, hb,
                                     rzb1_c, bin1_c, bhnd1_c)
                d1[0](0)
                d1[2]()
                h1b = d1_finish(d1)
                for t in range(PRED):
                    h1r = (lambda hh: lambda j: hh[:, j, :])(h1b)
                    wh2, _, ghn2, gxn2, chain2 = _dec_cell_parts(
                        nc, pls, 0, l2, KH, h1b, rzb2_c, bin2_c, bhnd2_c)
                    wh2(0)
                    ghn2()
                    gxn2(0, h1r)
                    gxn2(1, h1r)
                    wh2(1)
                    hb = chain2()
                    if t + 1 < PRED:
                        d1 = _dec_cell_parts(nc, pls, KE, l1h, KE, hb,
                                             rzb1_c, bin1_c, bhnd1_c)
                        d1[0](0)
                        d1[2]()
                    outf, outb = _emit_out_block(nc, wout2_s, bout_c, hb,
                                                 psXE, "pgxn", outp, preds, t)
                    if t + 1 < PRED:
                        emit_demb(outb)
                        h1b = d1_finish(d1)
            wE1.release()
    nc.compile()
    return nc


# ----------------------------------------------------------------------------
# host side
# ----------------------------------------------------------------------------

def _tiles(w):
    """(G, fin) weight -> transposed k-tiles (fin/128, 128, G) fp16."""
    wt = np.ascontiguousarray(w.T)
    return wt.reshape(-1, 128, w.shape[0]).astype(np.float16)


def _cols(v):
    """(n*128,) bias -> (128, n) f32 with [p, j] = v[j*128+p]."""
    return np.ascontiguousarray(v.reshape(-1, 128).T.astype(np.float32))


def _perm_rz_cols(wt, width):
    """Apply the P16 bank-interleave to the first 2048 of `width` columns
    of (k, 128, width) weight tiles."""
    P24 = [0, 1, 8, 9, 2, 3, 10, 11, 4, 5, 12, 13, 6, 7, 14, 15]
    kk = wt.shape[0]
    w4 = wt.reshape(kk, 128, width // 128, 128)
    perm = P24 + list(range(16, width // 128))
    return np.ascontiguousarray(w4[:, :, perm].reshape(kk, 128, width))


def _perm_row(v):
    """(2048,) r/z bias -> P16-interleaved row."""
    t = v.reshape(16, 128)
    P24 = [0, 1, 8, 9, 2, 3, 10, 11, 4, 5, 12, 13, 6, 7, 14, 15]
    return np.ascontiguousarray(t[P24].reshape(1, 2048))


def kernel(**inputs):
    ins = {k: np.asarray(v, np.float32) for k, v in inputs.items()}
    if "nc" not in _CACHE:
        _CACHE["nc"] = build_program()
    nc = _CACHE["nc"]

    w = {}
    w["WeT"] = np.ascontiguousarray(ins["We"].T).astype(np.float16)
    w["WedT"] = np.ascontiguousarray(ins["Wed"].T).astype(np.float16)

    w["Wih1T"] = _perm_rz_cols(_tiles(ins["enc1_Wih"]), G)
    w["Whh1T"] = _perm_rz_cols(_tiles(ins["enc1_Whh"]), G)
    w["Wih2T"] = _perm_rz_cols(_tiles(ins["enc2_Wih"]), G)
    w["Whh2T"] = _perm_rz_cols(_tiles(ins["enc2_Whh"]), G)
    w["Wd1T"] = _perm_rz_cols(np.concatenate(
        [_tiles(ins["dec1_Wih"]), _tiles(ins["dec1_Whh"])], axis=0), G)
    wi, wh = ins["dec2_Wih"], ins["dec2_Whh"]
    wd2 = np.concatenate(
        [np.ascontiguousarray((wi[:2 * H] + wh[:2 * H]).T),
         np.ascontiguousarray(wi[2 * H:].T),
         np.ascontiguousarray(wh[2 * H:].T)], axis=1)  # (H, 4096)
    w["Wd2T"] = _perm_rz_cols(
        wd2.reshape(KH, 128, 4096).astype(np.float16), 4096)
    w["WoutT"] = np.ascontiguousarray(ins["Wout"].T).reshape(
        KH, 128, 2).astype(np.float16)
    w["be_s"] = _cols(ins["be"])
    w["bed_s"] = _cols(ins["bed"])
    w["bout_s"] = ins["bout"].reshape(2, 1).astype(np.float32)
    P24 = [0, 1, 8, 9, 2, 3, 10, 11, 4, 5, 12, 13, 6, 7, 14, 15] + \
        list(range(16, 24))
    for pre, gq, bq in (("enc1", "gxb1", "bhn1_c"), ("enc2", "gxb2", "bhn2_c")):
        bih, bhh = ins[pre + "_bih"], ins[pre + "_bhh"]
        w[gq] = np.ascontiguousarray(_cols(np.concatenate(
            [bih[:2 * H] + bhh[:2 * H], bih[2 * H:]]))[:, P24])
        w[bq] = _cols(bhh[2 * H:])
    for pre, tag in (("dec1", "d1"), ("dec2", "d2")):
        bih, bhh = ins[pre + "_bih"], ins[pre + "_bhh"]
        w[tag + "_rzb_c"] = np.ascontiguousarray(
            _cols(bih[:2 * H] + bhh[:2 * H])[:, P16])
        w[tag + "_bin_c"] = _cols(bih[2 * H:])
        w[tag + "_bhn_c"] = _cols(bhh[2 * H:])

    obs = ins["obs"]
    in_maps = []
    for c in range(N_CORES):
        m = dict(w)
        ob = obs[c * BL:(c + 1) * BL]                  # (BL, T, 2)
        m["obsT"] = np.ascontiguousarray(
            ob.transpose(2, 1, 0)).reshape(2, T * BL).astype(np.float16)
        in_maps.append(m)

    _CACHE["in_maps"] = in_maps
    res = run_bass_kernel_spmd(nc, in_maps, list(range(N_CORES)))
    outs = []
    for c in range(N_CORES):
        p = res.results[c]["preds"]                    # (2, PRED, BL)
        outs.append(np.ascontiguousarray(p.transpose(2, 1, 0)))
    return np.concatenate(outs, axis=0).astype(np.float32)



# revision 46
# speedup vs baseline: 1.0473x; 1.0473x over previous
"""Trainium2 Bass kernel for the GRU encoder-decoder model.

Model (see harness reference): B=1024, T=100, PRED=30, E=512, H=1024, IN=2.
  emb = tanh(obs @ We.T + be)                      (B,T,512)
  enc1 = GRU(emb), enc2 = GRU(enc1, h0=h_enc1)     hidden 1024
  out0 = enc2[:,-2] @ Wout.T + bout
  30-step autoregressive decoder with two GRU cells sharing one hidden.

Strategy: data-parallel over batch on 8 cores (128 rows/core), all compute in
a feature-on-partition layout ([128 part = feature chunk, free = batch]),
weights pre-transposed on the host as the stationary fp16 matmul operand.

Key structure decisions (from trace analysis):
  * gx1 = emb@Wih1.T (+all biases) precomputed in a N=512 phase, DRAM fp16.
  * gx2 = h1@Wih2.T (+all biases) is fused INTO the enc1 scan: after each
    step's recurrence matmuls the tensor engine computes gx2 for the hidden
    just produced, filling the gate-chain bubble and keeping the PE warm.
    The PSUM->SBUF copy applies the gx2 biases (per-tile tensor_scalar_add
    on Vector, which has slack in that phase).
  * No bias matmuls anywhere: bhn is applied by a per-bank u = ghn + bhn
    copy (Scalar for the encoder scans, Vector tensor_scalar_add in the
    decoder) emitted as soon as the ghn PSUM bank closes; the decoder's
    rzb is the sigmoid's activation bias and bin rides a fused
    scalar_tensor_tensor in t2.
  * r/z matmuls run as two half-waves into a bufs=2 PSUM pool with the
    ghn block between them, so accumulation groups close mid-step and the
    next step's waves only wait on early sigmoid banks.
  * Decoder emission order: dec1(t+1)'s hidden-side waves + ghn are
    emitted before step t's out block + demb, covering the chain2 /
    out->demb serial path; dec1's x-side closes wave 0 first.
  * dec1's weights prefetch via DMA drip through phase D's steps.
  * Gate chain at hidden-chunk granularity, smallest chunk first, with the
    r/z sigmoids interleaved between chunks; the next step's r/z matmuls
    (k-outer) consume hidden chunks as they emerge.
  * dec2 reads the same vector for input and hidden, so its r/z input +
    hidden weights are pre-summed on the host (saves 1/3 of its matmuls).
"""

import numpy as np
import ml_dtypes

import concourse.bass as bass
import concourse.mybir as mybir
import concourse.tile as tile
from concourse import bacc
from concourse.bass_utils import run_bass_kernel_spmd

F32 = mybir.dt.float32
BF16 = mybir.dt.bfloat16
FP16 = mybir.dt.float16
AF = mybir.ActivationFunctionType
ALU = mybir.AluOpType

N_CORES = 8
B, T, PRED = 1024, 100, 30
E, H, IN = 512, 1024, 2
BL = B // N_CORES          # 128 batch rows per core
G = 3 * H                  # 3072 stacked gate rows
KH = H // 128              # 8 hidden k-tiles
KE = E // 128              # 4 embedding k-tiles
NT = T * BL // 512         # 25 n-chunks of 512 in the gx1 phase
# chain chunks over the 8 hidden tiles: smallest first so the next scan
# step's matmuls (which consume h k-tiles in ascending order) start early.
CHUNKS = [(0, 1), (1, 2), (2, 4), (4, 6), (6, 7), (7, 8)]
# r/z m-tiles bank-interleaved [r0 r1 z0 z1 | r2 r3 z2 z3 | ...] so the
# sigmoid of bank j//2 unlocks chain chunk j.
P16 = [0, 1, 8, 9, 2, 3, 10, 11, 4, 5, 12, 13, 6, 7, 14, 15]

_CACHE = {}


def _rz_sl(lo, hi, zgate):
    """rz_sb slice holding r (or z) tiles [lo,hi) in the perm layout."""
    idx = [4 * (j // 2) + 2 * zgate + (j % 2) for j in range(lo, hi)]
    assert idx == list(range(idx[0], idx[0] + len(idx))), idx
    return slice(idx[0], idx[0] + len(idx))


# ----------------------------------------------------------------------------
# device program
# ----------------------------------------------------------------------------

def _emit_gx1_phase(nc, tc, wk, rhs_fn, gx_dram, gxb_s, pools):
    """gx1' = Wih1.T-tiles @ emb (+bias) -> DRAM fp16, software-pipelined:
    the next chunk's emb matmuls+tanh are slotted mid-way through this
    chunk's gx matmuls so the tensor engine never waits on the tanh."""
    ps_gx = pools["ps_gx"]
    gxop = pools["gxo"]
    pend = [rhs_fn(0)]
    for c in range(NT):
        rhs = pend.pop(0)   # list of [128, 512] APs, one per k-tile
        for a in range(24):
            if a == 12 and c + 1 < NT:
                pend.append(rhs_fn(c + 1))
            pg = ps_gx.tile([128, 512], F32, tag="pgx")
            for j in range(KE):
                nc.tensor.matmul(
                    pg[:], wk[:, j, a * 128:(a + 1) * 128], rhs[j],
                    start=(j == 0), stop=(j == KE - 1))
            gxo = gxop.tile([128, 4, 128], FP16, tag="gxo")
            nc.scalar.activation(
                gxo.rearrange("p t b -> p (t b)"), pg[:], AF.Identity,
                bias=gxb_s[:, a:a + 1])
            nc.sync.dma_start(
                out=gx_dram[4 * c:4 * c + 4, a].rearrange("t p b -> p t b"),
                in_=gxo[:])


def _emit_out_block(nc, wout_s, bout_s, hb, ps_pool, ps_tag, outp,
                    preds=None, t=None):
    """outT = h @ Wout.T + bout -> ([2,128] f32, [2,128] fp16)."""
    po = ps_pool.tile([2, 128], F32, tag=ps_tag)
    for j in range(KH):
        nc.tensor.matmul(po[:], wout_s[:, j, :], hb[:, j, :],
                         start=(j == 0), stop=(j == KH - 1))
    outf = outp.tile([2, 128], F32, tag="outf")
    nc.vector.tensor_scalar_add(outf[:], po[:], bout_s[:, 0:1])
    outb = outp.tile([2, 128], FP16, tag="outb")
    nc.vector.tensor_copy(outb[:], outf[:])
    if preds is not None:
        nc.sync.dma_start(out=preds[:, t, :], in_=outf[:])
    return outf, outb


def _emit_chain(nc, pools, p_ghn, hb, rz_sb, bhn_c, t2_emit, sig_cbs,
                u_vec=False, upd_gp=False):
    """Gate math after the matmuls: returns the new fp16 hidden state.

    t1 = (ghn + bhn) * r is a single fused scalar_tensor_tensor per hidden
    tile (bhn_c is a [128, 8] f32 column table), so bhn never costs a PE
    matmul.  rz_sb is filled lazily via sig_cbs[bank] (emitted just before
    the first chunk that needs that bank, so the r/z sigmoids interleave
    with the tanh chain on the Scalar queue).
    t2_emit(lo, hi, t1, t2) -> emits t2 = t1 + x-projection for the chunk.
    """
    tmp = pools["tmp"]
    hbp = pools["hb"]
    u = tmp.tile([128, 8, 128], FP16, tag="u", bufs=1)
    t1 = tmp.tile([128, 8, 128], FP16, tag="t1")
    t2 = tmp.tile([128, 8, 128], FP16, tag="t2", bufs=1)
    nn = tmp.tile([128, 8, 128], FP16, tag="nn", bufs=1)
    d = tmp.tile([128, 8, 128], FP16, tag="d", bufs=1)
    hz = tmp.tile([128, 8, 128], FP16, tag="hz", bufs=1)
    nhb = hbp.tile([128, 8, 128], FP16, tag="hb")
    done = set()
    udone = set()
    # software-pipelined emission: chunk c's h-update (d/hz/nhb) is emitted
    # AFTER chunk c+1's t1/t2/tanh, so the in-order Vector queue never has a
    # tanh-dependent op blocking the next chunk's independent t1.
    # u = ghn + bhn runs on Scalar per ghn PSUM bank as soon as that bank
    # closes, so the p_ghn banks are drained mid-chain (not gated on the
    # late sigmoids) and the pool's next allocation never stalls the PE.
    for i in range(len(CHUNKS) + 1):
        if i < len(CHUNKS):
            lo, hi = CHUNKS[i]
            for bk in range(lo // 2, (hi + 1) // 2):
                if bk not in done:
                    sig_cbs(bk)
                    done.add(bk)
            for gb in {lo // 4, (hi - 1) // 4}:
                if gb not in udone:
                    for a in range(4 * gb, 4 * gb + 4):
                        if u_vec:
                            nc.vector.tensor_scalar_add(
                                u[:, a, :], p_ghn[:, a, :],
                                bhn_c[:, a:a + 1])
                        else:
                            nc.scalar.activation(u[:, a, :], p_ghn[:, a, :],
                                                 AF.Identity,
                                                 bias=bhn_c[:, a:a + 1])
                    udone.add(gb)
            sl = slice(lo, hi)
            nc.vector.tensor_mul(t1[:, sl, :], u[:, sl, :],
                                 rz_sb[:, _rz_sl(lo, hi, 0), :])
            t2_emit(lo, hi, t1, t2)
            nc.scalar.activation(nn[:, sl, :], t2[:, sl, :], AF.Tanh)
        if i > 0:
            lo, hi = CHUNKS[i - 1]
            sl = slice(lo, hi)
            upd = nc.gpsimd if upd_gp else nc.vector
            upd.tensor_sub(d[:, sl, :], hb[:, sl, :], nn[:, sl, :])
            upd.tensor_mul(hz[:, sl, :], d[:, sl, :],
                           rz_sb[:, _rz_sl(lo, hi, 1), :])
            upd.tensor_add(nhb[:, sl, :], hz[:, sl, :], nn[:, sl, :])
    return nhb


def _emit_enc_scan(nc, tc, pools, whh_s, gx_dram, bhn_c, hb,
                   out98=None, gx2=None, dmas=None):
    """Encoder scan over T steps.

    gx2 (scan1 fusion): dict(w=wih2_s, b=gxb2_s cols, ps=psum pool, op=sbuf
    pool, dram=gx2_dram) — after each step's recurrence matmuls, compute
    gx2[t-1] = Wih2.T @ h1[t-1] on the tensor engine (6 waves of 4 m-tiles;
    the PSUM->SBUF copy folds the gxb2 biases on the Scalar engine).
    The r/z matmuls run as two half-waves into a bufs=2 PSUM pool, so the
    next step's wave 0 only waits for this step's bank-0/1 sigmoids (early
    in the chain) instead of all four.  bhn is applied in the chain's fused
    scalar_tensor_tensor — no bias matmuls anywhere.
    Returns (hb, out_res at t = T-2 if out98 given)."""
    gxp = pools["gx"]
    tmp = pools["tmp"]
    ps_rz, ps_ghn = pools["ps_rz"], pools["ps_ghn"]
    PF = 2
    pend = []
    for t in range(min(PF, T)):
        gxt = gxp.tile([128, 24, 128], FP16, tag="gxt")
        nc.sync.dma_start(out=gxt[:], in_=gx_dram[t].rearrange("a p b -> p a b"))
        pend.append(gxt)

    def emit_gx2(h_in, td):
        """gx2[td] = Wih2.T @ h1[td] + gxb2 -> DRAM (6 waves x 4 m-tiles)."""
        for w in range(6):
            pg = gx2["ps"].tile([128, 4, 128], F32, tag="pgx2")
            for j in range(KH):
                for a in range(4):
                    nc.tensor.matmul(
                        pg[:, a, :],
                        gx2["w"][:, j, (4 * w + a) * 128:(4 * w + a + 1) * 128],
                        h_in[:, j, :], start=(j == 0 and a == 0),
                        stop=(j == KH - 1 and a == 3))
            go = gx2["op"].tile([128, 4, 128], FP16, tag="gxo2")
            for a in range(4):
                nc.scalar.activation(go[:, a, :], pg[:, a, :], AF.Identity,
                                     bias=gx2["b"][:, 4 * w + a:4 * w + a + 1])
            nc.sync.dma_start(
                out=gx2["dram"][td, 4 * w:4 * w + 4].rearrange("a p b -> p a b"),
                in_=go[:])

    out_res = None
    for t in range(T):
        gxt = pend.pop(0)
        if dmas and t >= 4 and t % 2 == 0:
            dmas.pop(0)()
        h_in = hb
        rzs = tmp.tile([128, 16, 128], FP16, tag="rzs", bufs=1)
        rz_sb = tmp.tile([128, 16, 128], FP16, tag="rz")
        # k-outer so each MM burst consumes one hidden chunk as the previous
        # step's chain emits it; two half-waves of 8 m-tiles (2 PSUM banks
        # each, double-buffered) so wave 0 of step t+1 only waits on this
        # step's first two sigmoid banks.
        # emission order [w0][ghn][w1]: ghn's bank 0 closes mid-step, so the
        # chain (whose t1 reads are gated on the ghn accumulation group
        # CLOSING, not just the tiles being written) starts ~5us earlier and
        # its nhb chunks are ready before the next step's w0 bursts need
        # them — removes the exposed chain tail in the enc2 phase.
        w_tiles = []

        def rz_wave(w):
            pw = ps_rz.tile([128, 8, 128], F32, tag="prz")
            for j in range(KH):
                for ai in range(8):
                    a = 8 * w + ai
                    nc.tensor.matmul(
                        pw[:, ai, :], whh_s[:, j, a * 128:(a + 1) * 128],
                        hb[:, j, :], start=(j == 0 and ai % 4 == 0),
                        stop=(j == KH - 1 and ai % 4 == 3))
            w_tiles.append(pw)

        rz_wave(0)
        p_ghn = ps_ghn.tile([128, 8, 128], F32, tag="pghn")
        for a in range(8):
            for j in range(KH):
                nc.tensor.matmul(
                    p_ghn[:, a, :], whh_s[:, j, (16 + a) * 128:(17 + a) * 128],
                    hb[:, j, :], start=(a % 4 == 0 and j == 0),
                    stop=(a % 4 == 3 and j == KH - 1))
        rz_wave(1)

        def sig(bk, w_tiles=w_tiles, gxt=gxt, rzs=rzs, rz_sb=rz_sb):
            sl = slice(4 * bk, 4 * bk + 4)
            psl = slice(4 * (bk % 2), 4 * (bk % 2) + 4)
            nc.vector.tensor_add(rzs[:, sl, :], w_tiles[bk // 2][:, psl, :],
                                 gxt[:, sl, :])
            nc.scalar.activation(rz_sb[:, sl, :], rzs[:, sl, :], AF.Sigmoid)

        def t2_enc(lo, hi, t1, t2, gxt=gxt):
            nc.vector.tensor_add(t2[:, lo:hi, :], t1[:, lo:hi, :],
                                 gxt[:, 16 + lo:16 + hi, :])

        hb = _emit_chain(nc, pools, p_ghn, hb, rz_sb, bhn_c, t2_enc, sig)
        if gx2 is not None and t > 0:
            emit_gx2(h_in, t - 1)
        if out98 is not None and t == T - 2:
            wout_s, bout_s, outp = out98
            out_res = _emit_out_block(nc, wout_s, bout_s, hb, ps_ghn, "pghn",
                                      outp)
        if t + PF < T:
            gxt2 = gxp.tile([128, 24, 128], FP16, tag="gxt")
            nc.sync.dma_start(out=gxt2[:],
                              in_=gx_dram[t + PF].rearrange("a p b -> p a b"))
            pend.append(gxt2)
    if gx2 is not None:
        emit_gx2(hb, T - 1)
    return hb, out_res


def _dec_cell_parts(nc, pools, nkx, lhsT_fn, xn_ks, hb,
                    rzb_c, bin_c, bhn_c):
    """Emitter closures for one decoder GRU cell; the caller controls the
    PE emission order so PSUM accumulation groups CLOSE as early as
    possible (chain reads are gated on the group's stop matmul, not on the
    tiles being written) and so independent matmul blocks cover the serial
    out->demb->x path.  No bias matmuls: rzb is the sigmoid's activation
    bias, bin rides the fused t2 scalar_tensor_tensor, bhn the fused t1."""
    tmp = pools["tmp"]
    ps_rz, ps_ghn, ps_x = pools["ps_rz"], pools["ps_ghn"], pools["ps_gxn"]
    st = {"w": [], "gxn": None}

    def rz_wave_h(w):
        pw = ps_rz.tile([128, 8, 128], F32, tag="prz")
        st["w"].append(pw)
        for ji in range(KH):
            for ai in range(8):
                a = 8 * w + ai
                nc.tensor.matmul(pw[:, ai, :], lhsT_fn("rz", nkx + ji, a),
                                 hb[:, ji, :],
                                 start=(ji == 0 and ai % 4 == 0),
                                 stop=(nkx == 0 and ji == KH - 1 and ai % 4 == 3))

    def rz_wave_x(w, xrhs):
        pw = st["w"][w]
        for xi in range(nkx):
            for ai in range(8):
                a = 8 * w + ai
                nc.tensor.matmul(pw[:, ai, :], lhsT_fn("rz", xi, a),
                                 xrhs(xi), start=False,
                                 stop=(xi == nkx - 1 and ai % 4 == 3))

    def ghn():
        p_ghn = ps_ghn.tile([128, 8, 128], F32, tag="pghn")
        st["ghn"] = p_ghn
        for a in range(8):
            for j in range(KH):
                nc.tensor.matmul(p_ghn[:, a, :], lhsT_fn("hn", j, a),
                                 hb[:, j, :], start=(a % 4 == 0 and j == 0),
                                 stop=(a % 4 == 3 and j == KH - 1))

    def gxn_half(h, xn_rhs):
        if st["gxn"] is None:
            p_gxn = ps_x.tile([128, 8, 128], F32, tag="pgxn")
            st["gxn"] = p_gxn
        p_gxn = st["gxn"]
        for j in range(xn_ks):
            for a in range(4 * h, 4 * h + 4):
                nc.tensor.matmul(p_gxn[:, a, :], lhsT_fn("xn", j, a),
                                 xn_rhs(j), start=(j == 0 and a == 4 * h),
                                 stop=(j == xn_ks - 1 and a == 4 * h + 3))

    def chain():
        rz_sb = tmp.tile([128, 16, 128], FP16, tag="rz")

        def sig(bk):
            pw = st["w"][bk // 2]
            for i in range(4):
                a = 4 * bk + i
                nc.scalar.activation(rz_sb[:, a, :], pw[:, 4 * (bk % 2) + i, :],
                                     AF.Sigmoid, bias=rzb_c[:, a:a + 1])

        def t2_dec(lo, hi, t1, t2):
            for a in range(lo, hi):
                nc.vector.scalar_tensor_tensor(
                    t2[:, a, :], st["gxn"][:, a, :], bin_c[:, a:a + 1],
                    t1[:, a, :], ALU.add, ALU.add)

        return _emit_chain(nc, pools, st["ghn"], hb, rz_sb, bhn_c, t2_dec,
                           sig, u_vec=True)

    return rz_wave_h, rz_wave_x, ghn, gxn_half, chain


def build_program():
    nc = bacc.Bacc("TRN2", target_bir_lowering=False, debug=False,
                   num_devices=N_CORES)
    dp = nc.declare_dram_parameter
    obsT = dp("obsT", [2, T * BL], FP16, isOutput=False)
    WeT = dp("WeT", [2, E], FP16, isOutput=False)
    Wih1T = dp("Wih1T", [KE, 128, G], FP16, isOutput=False)
    Whh1T = dp("Whh1T", [KH, 128, G], FP16, isOutput=False)
    Wih2T = dp("Wih2T", [KH, 128, G], FP16, isOutput=False)
    Whh2T = dp("Whh2T", [KH, 128, G], FP16, isOutput=False)
    Wd1T = dp("Wd1T", [KE + KH, 128, G], FP16, isOutput=False)
    Wd2T = dp("Wd2T", [KH, 128, 4096], FP16, isOutput=False)
    WedT = dp("WedT", [2, E], FP16, isOutput=False)
    WoutT = dp("WoutT", [KH, 128, 2], FP16, isOutput=False)
    be_s = dp("be_s", [128, KE], F32, isOutput=False)
    gxb1 = dp("gxb1", [128, 24], F32, isOutput=False)
    gxb2 = dp("gxb2", [128, 24], F32, isOutput=False)
    bhn1_d = dp("bhn1_c", [128, KH], F32, isOutput=False)
    bhn2_d = dp("bhn2_c", [128, KH], F32, isOutput=False)
    d1_rzb_d = dp("d1_rzb_c", [128, 16], F32, isOutput=False)
    d1_bin_d = dp("d1_bin_c", [128, KH], F32, isOutput=False)
    d1_bhn_d = dp("d1_bhn_c", [128, KH], F32, isOutput=False)
    d2_rzb_d = dp("d2_rzb_c", [128, 16], F32, isOutput=False)
    d2_bin_d = dp("d2_bin_c", [128, KH], F32, isOutput=False)
    d2_bhn_d = dp("d2_bhn_c", [128, KH], F32, isOutput=False)
    bed_s = dp("bed_s", [128, KE], F32, isOutput=False)
    bout_s = dp("bout_s", [2, 1], F32, isOutput=False)
    preds = dp("preds", [2, PRED, BL], F32, isOutput=True)

    gx1 = nc.dram_tensor("gx1", [T, 24, 128, BL], FP16)
    gx2 = nc.dram_tensor("gx2", [T, 24, 128, BL], FP16)

    with tile.TileContext(nc) as tc:
        with tc.tile_pool(name="const", bufs=1) as constp, \
             tc.tile_pool(name="hbp", bufs=2) as hbp, \
             tc.tile_pool(name="outp", bufs=2) as outp:
            def cload(name, ap, shape, dtype=F32):
                t = constp.tile(shape, dtype, tag=name)
                nc.sync.dma_start(out=t[:], in_=ap)
                return t
            gxb1_s = cload("gxb1", gxb1[:], [128, 24])
            gxb2_s = cload("gxb2", gxb2[:], [128, 24])
            be_c = cload("be", be_s[:], [128, KE])
            bout_c = cload("bout", bout_s[:], [2, 1])
            bhn1_c = cload("bhn1c", bhn1_d[:], [128, KH])
            bhn2_c = cload("bhn2c", bhn2_d[:], [128, KH])

            pools = {"hb": hbp}

            # weights for the fused scan1 phase; loaded up-front so the DMAs
            # overlap phase A compute.
            with tc.tile_pool(name="wB", bufs=1) as wB:
                whh1_s = wB.tile([128, KH, G], FP16)
                for j in range(KH):
                    nc.sync.dma_start(out=whh1_s[:, j, :], in_=Whh1T[j])
                wih2_s = wB.tile([128, KH, G], FP16)
                for j in range(KH):
                    nc.sync.dma_start(out=wih2_s[:, j, :], in_=Wih2T[j])

                # ---------------- phase A: emb + gx1 ----------------
                with tc.tile_pool(name="wA", bufs=1) as wA, \
                     tc.tile_pool(name="sA", bufs=2) as sA, \
                     tc.tile_pool(name="embp", bufs=2) as embp, \
                     tc.tile_pool(name="gxoA", bufs=4) as gxoA, \
                     tc.tile_pool(name="psE", bufs=1, space="PSUM") as psE, \
                     tc.tile_pool(name="psGA", bufs=4, space="PSUM") as psGA:
                    weT_s = wA.tile([2, E], FP16)
                    nc.sync.dma_start(out=weT_s[:], in_=WeT[:])
                    wih1_s = wA.tile([128, KE, G], FP16)
                    for j in range(KE):
                        nc.sync.dma_start(out=wih1_s[:, j, :], in_=Wih1T[j])

                    # obs streamed in 5 double-buffered pieces of 5 chunks
                    # each (the whole [2, T*BL] tile would cost 25.6K/part).
                    obs_box = {}

                    def load_obs(i):
                        ot = sA.tile([2, 5 * 512], FP16, tag="obs")
                        nc.sync.dma_start(
                            out=ot[:], in_=obsT[:, i * 2560:(i + 1) * 2560])
                        obs_box[i] = ot

                    load_obs(0)

                    def rhs_emb(c):
                        i = c // 5
                        if c % 5 == 0 and i + 1 < 5 and i + 1 not in obs_box:
                            load_obs(i + 1)
                        obs_s = obs_box[i]
                        co = (c % 5) * 512
                        pe = psE.tile([128, 2, 512], F32, tag="pemb")
                        for et in range(2):
                            nc.tensor.matmul(
                                pe[:, et, :], weT_s[:, et * 128:(et + 1) * 128],
                                obs_s[:, co:co + 512],
                                start=True, stop=True)
                        pe2 = psE.tile([128, 2, 512], F32, tag="pemb2")
                        for et in range(2):
                            nc.tensor.matmul(
                                pe2[:, et, :], weT_s[:, (2 + et) * 128:(3 + et) * 128],
                                obs_s[:, co:co + 512],
                                start=True, stop=True)
                        embb = embp.tile([128, KE, 512], FP16, tag="emb")
                        for et in range(KE):
                            src = pe[:, et, :] if et < 2 else pe2[:, et - 2, :]
                            nc.scalar.activation(embb[:, et, :], src,
                                                 AF.Tanh, bias=be_c[:, et:et + 1])
                        return [embb[:, et, :] for et in range(KE)]
                    _emit_gx1_phase(nc, tc, wih1_s, rhs_emb, gx1, gxb1_s,
                                    {"ps_gx": psGA, "gxo": gxoA})

                # ---------------- phase B: enc1 scan + fused gx2 ------------
                # whh2 prefetches during B on the right-side pool stack
                # (independent LIFO), its k-tile DMAs dripped between steps.
                wDpre = tc.alloc_tile_pool(name="wDpre", bufs=1, side="right")
                whh2_s = wDpre.tile([128, KH, G], FP16)
                wout_s = wDpre.tile([128, KH, 2], FP16)
                dmasB = [
                    (lambda j=j: nc.sync.dma_start(out=whh2_s[:, j, :],
                                                   in_=Whh2T[j]))
                    for j in range(KH)]
                dmasB.append(lambda: nc.sync.dma_start(
                    out=wout_s[:], in_=WoutT.ap().rearrange("j p m -> p j m")))
                hb = hbp.tile([128, KH, 128], FP16, tag="hb")
                nc.vector.memset(hb[:], 0.0)
                with tc.tile_pool(name="gxB", bufs=3) as gxB, \
                     tc.tile_pool(name="tmpB", bufs=2) as tmpB, \
                     tc.tile_pool(name="gxo2", bufs=2) as gxo2, \
                     tc.tile_pool(name="psRZ", bufs=2, space="PSUM") as psRZ, \
                     tc.tile_pool(name="psGH", bufs=1, space="PSUM") as psGH, \
                     tc.tile_pool(name="psGX2", bufs=2, space="PSUM") as psGX2:
                    pls = dict(pools, gx=gxB, tmp=tmpB, ps_rz=psRZ, ps_ghn=psGH)
                    g2 = {"w": wih2_s, "b": gxb2_s, "ps": psGX2, "op": gxo2,
                          "dram": gx2}
                    hb, _ = _emit_enc_scan(nc, tc, pls, whh1_s, gx1, bhn1_c,
                                           hb, gx2=g2, dmas=dmasB)

            # ---------------- phase D: enc2 scan (+ out_loc0 at t=98) -------
            # dec1's weights + the small decoder constants load during D
            # (wE1 outlives the D pools; LIFO: released after phase E).
            wE1 = tc.alloc_tile_pool(name="wE1", bufs=1)
            wd1_s = wE1.tile([128, KE + KH, G], FP16)
            dmasD = [
                (lambda j=j: nc.sync.dma_start(out=wd1_s[:, j, :],
                                               in_=Wd1T[j]))
                for j in range(KE + KH)]
            wed_s = wE1.tile([2, E], FP16)
            nc.sync.dma_start(out=wed_s[:], in_=WedT[:])
            wout2_s = wE1.tile([128, KH, 2], FP16)
            nc.sync.dma_start(out=wout2_s[:],
                              in_=WoutT.ap().rearrange("j p m -> p j m"))
            rzb1_c = wE1.tile([128, 16], F32)
            nc.sync.dma_start(out=rzb1_c[:], in_=d1_rzb_d[:])
            bin1_c = wE1.tile([128, KH], F32)
            nc.sync.dma_start(out=bin1_c[:], in_=d1_bin_d[:])
            bhnd1_c = wE1.tile([128, KH], F32)
            nc.sync.dma_start(out=bhnd1_c[:], in_=d1_bhn_d[:])
            rzb2_c = wE1.tile([128, 16], F32)
            nc.sync.dma_start(out=rzb2_c[:], in_=d2_rzb_d[:])
            bin2_c = wE1.tile([128, KH], F32)
            nc.sync.dma_start(out=bin2_c[:], in_=d2_bin_d[:])
            bhnd2_c = wE1.tile([128, KH], F32)
            nc.sync.dma_start(out=bhnd2_c[:], in_=d2_bhn_d[:])
            bed_c = wE1.tile([128, KE], F32)
            nc.sync.dma_start(out=bed_c[:], in_=bed_s[:])
            with tc.tile_pool(name="gxD", bufs=3) as gxD, \
                 tc.tile_pool(name="tmpD", bufs=2) as tmpD, \
                 tc.tile_pool(name="psRZD", bufs=2, space="PSUM") as psRZD, \
                 tc.tile_pool(name="psGHD", bufs=2, space="PSUM") as psGHD:
                pls = dict(pools, gx=gxD, tmp=tmpD, ps_rz=psRZD, ps_ghn=psGHD)
                hb, out_res = _emit_enc_scan(
                    nc, tc, pls, whh2_s, gx2, bhn2_c, hb,
                    out98=(wout_s, bout_c, outp), dmas=dmasD)
                outf, outb = out_res
            wDpre.release()

            # ---------------- phase E: decoder ----------------
            with tc.tile_pool(name="wE2", bufs=1) as wE2, \
                 tc.tile_pool(name="dembp", bufs=2) as dembp, \
                 tc.tile_pool(name="tmpE", bufs=2) as tmpE, \
                 tc.tile_pool(name="psRZE", bufs=2, space="PSUM") as psRZE, \
                 tc.tile_pool(name="psGHE", bufs=1, space="PSUM") as psGHE, \
                 tc.tile_pool(name="psXE", bufs=1, space="PSUM") as psXE:
                wd2_s = wE2.tile([128, KH, 4096], FP16)
                for j in range(KH):
                    nc.sync.dma_start(out=wd2_s[:, j, :], in_=Wd2T[j])

                pls = dict(pools, tmp=tmpE, ps_rz=psRZE, ps_ghn=psGHE,
                           ps_gxn=psXE)
                demb_box = [None]

                def emit_demb(ob):
                    """demb = tanh(Wed @ out + bed), from the fp16 out."""
                    p_de = psXE.tile([128, KE, 128], F32, tag="pgxn")
                    for et in range(KE):
                        nc.tensor.matmul(
                            p_de[:, et, :], wed_s[:, et * 128:(et + 1) * 128],
                            ob[:], start=(et == 0), stop=(et == KE - 1))
                    demb = dembp.tile([128, KE, 128], FP16, tag="demb")
                    for et in range(KE):
                        nc.scalar.activation(demb[:, et, :], p_de[:, et, :],
                                             AF.Tanh, bias=bed_c[:, et:et + 1])
                    demb_box[0] = demb

                def l1h(part, j, a):
                    if part == "hn":
                        return wd1_s[:, KE + j, (16 + a) * 128:(17 + a) * 128]
                    m = a if part == "rz" else 16 + a
                    return wd1_s[:, j, m * 128:(m + 1) * 128]

                def l2(part, j, a):
                    off = {"rz": a * 128, "xn": 2048 + a * 128,
                           "hn": 3072 + a * 128}[part]
                    return wd2_s[:, j, off:off + 128]

                def d1_finish(d1):
                    """dec1 x-side: wave 0 closes first so sigmoid banks 0/1
                    and the chain start while gxn + wave 1 still stream."""
                    wh, wx, _, gxn, chain = d1
                    dembr = (lambda dd: lambda j: dd[:, j, :])(demb_box[0])
                    wh(1)
                    wx(0, dembr)
                    gxn(0, dembr)
                    gxn(1, dembr)
                    wx(1, dembr)
                    return chain()

                # loop structure per step: dec2's gxn banks close early (they
                # gate chain2's t2); dec1(t+1)'s w0h+ghn are emitted before
                # the out block + demb so the PE queue has cover while
                # chain2(t) resolves; dec1's w1h covers the demb tanh; the
                # x-side waves close wave 0 first to start chain1 early.
                emit_demb(outb)
                d1 = _dec_cell_parts(nc, pls, KE, l1h, KE, hb,
                                     rzb1_c, bin1_c, bhnd1_c)
                d1[0](0)
                d1[2]()
                h1b = d1_finish(d1)
                for t in range(PRED):
                    h1r = (lambda hh: lambda j: hh[:, j, :])(h1b)
                    wh2, _, ghn2, gxn2, chain2 = _dec_cell_parts(
                        nc, pls, 0, l2, KH, h1b, rzb2_c, bin2_c, bhnd2_c)
                    wh2(0)
                    ghn2()
                    gxn2(0, h1r)
                    gxn2(1, h1r)
                    wh2(1)
                    hb = chain2()
                    if t + 1 < PRED:
                        d1 = _dec_cell_parts(nc, pls, KE, l1h, KE, hb,
                                             rzb1_c, bin1_c, bhnd1_c)
                        d1[0](0)
                        d1[2]()
                    outf, outb = _emit_out_block(nc, wout2_s, bout_c, hb,
                                                 psXE, "pgxn", outp, preds, t)
                    if t + 1 < PRED:
                        emit_demb(outb)
                        h1b = d1_finish(d1)
            wE1.release()
    nc.compile()
    return nc


# ----------------------------------------------------------------------------
# host side
# ----------------------------------------------------------------------------

def _tiles(w):
    """(G, fin) weight -> transposed k-tiles (fin/128, 128, G) fp16."""
    wt = np.ascontiguousarray(w.T)
    return wt.reshape(-1, 128, w.shape[0]).astype(np.float16)


def _cols(v):
    """(n*128,) bias -> (128, n) f32 with [p, j] = v[j*128+p]."""
    return np.ascontiguousarray(v.reshape(-1, 128).T.astype(np.float32))


def _perm_rz_cols(wt, width):
    """Apply the P16 bank-interleave to the first 2048 of `width` columns
    of (k, 128, width) weight tiles."""
    P24 = [0, 1, 8, 9, 2, 3, 10, 11, 4, 5, 12, 13, 6, 7, 14, 15]
    kk = wt.shape[0]
    w4 = wt.reshape(kk, 128, width // 128, 128)
    perm = P24 + list(range(16, width // 128))
    return np.ascontiguousarray(w4[:, :, perm].reshape(kk, 128, width))


def _perm_row(v):
    """(2048,) r/z bias -> P16-interleaved row."""
    t = v.reshape(16, 128)
    P24 = [0, 1, 8, 9, 2, 3, 10, 11, 4, 5, 12, 13, 6, 7, 14, 15]
    return np.ascontiguousarray(t[P24].reshape(1, 2048))


def kernel(**inputs):
    ins = {k: np.asarray(v, np.float32) for k, v in inputs.items()}
    if "nc" not in _CACHE:
        _CACHE["nc"] = build_program()
    nc = _CACHE["nc"]

    w = {}
    w["WeT"] = np.ascontiguousarray(ins["We"].T).astype(np.float16)
    w["WedT"] = np.ascontiguousarray(ins["Wed"].T).astype(np.float16)

    w["Wih1T"] = _perm_rz_cols(_tiles(ins["enc1_Wih"]), G)
    w["Whh1T"] = _perm_rz_cols(_tiles(ins["enc1_Whh"]), G)
    w["Wih2T"] = _perm_rz_cols(_tiles(ins["enc2_Wih"]), G)
    w["Whh2T"] = _perm_rz_cols(_tiles(ins["enc2_Whh"]), G)
    w["Wd1T"] = _perm_rz_cols(np.concatenate(
        [_tiles(ins["dec1_Wih"]), _tiles(ins["dec1_Whh"])], axis=0), G)
    wi, wh = ins["dec2_Wih"], ins["dec2_Whh"]
    wd2 = np.concatenate(
        [np.ascontiguousarray((wi[:2 * H] + wh[:2 * H]).T),
         np.ascontiguousarray(wi[2 * H:].T),
         np.ascontiguousarray(wh[2 * H:].T)], axis=1)  # (H, 4096)
    w["Wd2T"] = _perm_rz_cols(
        wd2.reshape(KH, 128, 4096).astype(np.float16), 4096)
    w["WoutT"] = np.ascontiguousarray(ins["Wout"].T).reshape(
        KH, 128, 2).astype(np.float16)
    w["be_s"] = _cols(ins["be"])
    w["bed_s"] = _cols(ins["bed"])
    w["bout_s"] = ins["bout"].reshape(2, 1).astype(np.float32)
    P24 = [0, 1, 8, 9, 2, 3, 10, 11, 4, 5, 12, 13, 6, 7, 14, 15] + \
        list(range(16, 24))
    for pre, gq, bq in (("enc1", "gxb1", "bhn1_c"), ("enc2", "gxb2", "bhn2_c")):
        bih, bhh = ins[pre + "_bih"], ins[pre + "_bhh"]
        w[gq] = np.ascontiguousarray(_cols(np.concatenate(
            [bih[:2 * H] + bhh[:2 * H], bih[2 * H:]]))[:, P24])
        w[bq] = _cols(bhh[2 * H:])
    for pre, tag in (("dec1", "d1"), ("dec2", "d2")):
        bih, bhh = ins[pre + "_bih"], ins[pre + "_bhh"]
        w[tag + "_rzb_c"] = np.ascontiguousarray(
            _cols(bih[:2 * H] + bhh[:2 * H])[:, P16])
        w[tag + "_bin_c"] = _cols(bih[2 * H:])
        w[tag + "_bhn_c"] = _cols(bhh[2 * H:])

    obs = ins["obs"]
    in_maps = []
    for c in range(N_CORES):
        m = dict(w)
        ob = obs[c * BL:(c + 1) * BL]                  # (BL, T, 2)
        m["obsT"] = np.ascontiguousarray(
            ob.transpose(2, 1, 0)).reshape(2, T * BL).astype(np.float16)
        in_maps.append(m)

    _CACHE["in_maps"] = in_maps
    res = run_bass_kernel_spmd(nc, in_maps, list(range(N_CORES)))
    outs = []
    for c in range(N_CORES):
        p = res.results[c]["preds"]                    # (2, PRED, BL)
        outs.append(np.ascontiguousarray(p.transpose(2, 1, 0)))
    return np.concatenate(outs, axis=0).astype(np.float32)

